# revision 19
# baseline (speedup 1.0000x reference)
"""Trainium2 Bass/Tile kernel for MAB-style attention block (nn_MAB_channel_aware_force).

Reference computation (per batch b of 32):
  q = Q @ Wq + bq ; k = K @ Wk + bk ; v = K @ Wv + bv          # [512, 512]
  per head h (8 heads, dh=64):
    scores = qh @ kh^T / sqrt(512) ; A = softmax(scores)
    oh = qh + A @ vh
  O = LN0(concat(oh)) ; O = O + relu(O @ Wo + bo) ; out = LN1(O)

Sharding: data-parallel over batch across 8 NeuronCores (4 batches/core).

v4 structure per core:
  - attention (C) per head-pair: per ki, both heads' score matmuls land in one
    [128, 2, 512] PSUM tile (adjacent in the PE queue), one exp per ki covers
    both heads, then both heads' A@V accumulations for that ki.  scp is
    double-buffered so scores(ki+1) overlap exp(ki); C is ACT-bound.
  - three batches in flight: C(b) emission interleaves the D/E/F/E2 tail of
    batch b-1 AND the A/B/qnat prep of batch b+1, so the ACT queue never
    head-of-line blocks the next batch's exps behind LN-stats ops.
  - attn^T + softmax sums drained in one DVE copy per head ([65, 512] bf16,
    sums row 64); sums DMA straight from SBUF; relu+residual fused in one
    scalar_tensor_tensor; DMAs coalesced (weights 1/matrix, inputs 2/batch,
    output 1/batch) to relieve the HWDGE issue queue.
"""

import numpy as np

import bass_rust as _bass_rust
import concourse.bass as bass
import concourse.mybir as mybir
import concourse.tile as tile
from concourse import bacc
from concourse.bass_utils import run_bass_kernel_spmd
from concourse.hw_specs import get_activation_tables
from concourse.masks import make_identity


class _BaccOneActTable(bacc.Bacc):
    """Bacc whose act-table pass is pinned to natural_log_exp_and_others.

    The stock greedy pass maps exp -> exp_and_others and ln -> natural_log
    (first set containing each function), forcing ~2.6us of ACT table
    reloads around every LayerNorm rsqrt (ln+exp) and again before the
    next softmax exp.  Every activation this kernel uses (exp, ln, copy,
    identity, relu) lives in the combined natural_log_exp_and_others set,
    so restricting the pass to that set yields exactly one table load.
    Set ids stay aligned with act_info.json (only the contents offered to
    the chooser are masked)."""

    _ACT_SET = "natural_log_exp_and_others"

    def insert_act_table_loads(self):
        has_activation = any(
            isinstance(i, mybir.InstActivation)
            for b in self.main_func.blocks
            for i in b.instructions
        )
        if not has_activation:
            return
        tables = [
            (name, (fns if name == self._ACT_SET else set()))
            for name, fns in get_activation_tables(self.m.arch).items()
        ]
        _bass_rust.insert_act_table_loads(self, tables)

P = 128
S = 512          # sequence length (Sq == Sk)
D = 512          # model dim == DIM_Q == DIM_K == DIM_V
H = 8            # heads
DH = D // H      # 64
NB = 4           # batches per core
NCORES = 8
EPS = 1e-5
SC = 1.0 / float(np.sqrt(D))
F32 = mybir.dt.float32
BF16 = mybir.dt.bfloat16
AF = mybir.ActivationFunctionType
OP = mybir.AluOpType

NBLK = S // P    # 4 sequence blocks of 128
NDB = D // P     # 4 feature blocks of 128


def build_program(zero_bias: bool, unit_ln: bool):
    nc = _BaccOneActTable("TRN2", target_bir_lowering=False, debug=False)

    Qd = nc.declare_dram_parameter("Q", [NB, S, D], F32, isOutput=False)
    Kd = nc.declare_dram_parameter("K", [NB, S, D], F32, isOutput=False)
    Wd = {}
    for w in ("Wq", "Wk", "Wv", "Wo"):
        Wd[w] = nc.declare_dram_parameter(w, [D, D], F32, isOutput=False)
    Bd = {}
    for v in ("bq", "bk", "bv", "bo", "ln0_g", "ln0_b", "ln1_g", "ln1_b"):
        Bd[v] = nc.declare_dram_parameter(v, [D], F32, isOutput=False)
    Od = nc.declare_dram_parameter("out", [NB, S, D], F32, isOutput=True)

    with tile.TileContext(nc) as tc:
        _build(nc, tc, Qd, Kd, Wd, Bd, Od, zero_bias, unit_ln)
    nc.compile()
    return nc


def _build(nc, tc, Qd, Kd, Wd, Bd, Od, zero_bias, unit_ln):
    from contextlib import ExitStack

    ctx = ExitStack()
    with ctx:
        const = ctx.enter_context(tc.tile_pool(name="const", bufs=1))
        stage = ctx.enter_context(tc.tile_pool(name="stage", bufs=2))
        loadp = ctx.enter_context(tc.tile_pool(name="loadp", bufs=4))
        n16p = ctx.enter_context(tc.tile_pool(name="n16p", bufs=5))
        t16p = ctx.enter_context(tc.tile_pool(name="t16p", bufs=12))
        projp = ctx.enter_context(tc.tile_pool(name="projp", bufs=17))
        vaugp = ctx.enter_context(tc.tile_pool(name="vaugp", bufs=9))
        qnatp = ctx.enter_context(tc.tile_pool(name="qnatp", bufs=9))
        expp = ctx.enter_context(tc.tile_pool(name="expp", bufs=2))
        atp = ctx.enter_context(tc.tile_pool(name="atp", bufs=10))
        rnp = ctx.enter_context(tc.tile_pool(name="rnp", bufs=6))
        ohp = ctx.enter_context(tc.tile_pool(name="ohp", bufs=5))
        ln0p = ctx.enter_context(tc.tile_pool(name="ln0p", bufs=5))
        lntp = ctx.enter_context(tc.tile_pool(name="lntp", bufs=5))
        p2p = ctx.enter_context(tc.tile_pool(name="p2p", bufs=5))
        outp = ctx.enter_context(tc.tile_pool(name="outp", bufs=1))
        statp = ctx.enter_context(tc.tile_pool(name="statp", bufs=10))

        dramp = ctx.enter_context(tc.tile_pool(name="dramp", bufs=3, space="DRAM"))
        # PSUM (8 banks): scores pairs 2x[2 banks], attn-out 2x[1], misc
        # (proj/fc/transposes) 2x[1].
        scp = ctx.enter_context(tc.tile_pool(name="scp", bufs=2, space="PSUM"))
        pop = ctx.enter_context(tc.tile_pool(name="pop", bufs=2, space="PSUM"))
        mp = ctx.enter_context(tc.tile_pool(name="mp", bufs=2, space="PSUM"))

        # ---- one-time constants ----
        I128b = const.tile([P, P], BF16)
        make_identity(nc, I128b)
        epsT = const.tile([P, 1], F32)
        nc.vector.memset(epsT[:], EPS)

        W16 = {}

        def emit_weight_load(w, wi):
            W16[w] = const.tile([P, NDB, D], BF16, tag=f"w16_{w}", name=f"w16_{w}")
            st = loadp.tile([P, NDB, D], F32, tag="wld", name="wld", bufs=2)
            nc.sync.dma_start(st[:], Wd[w].ap().rearrange("(o p) n -> p o n", p=P))
            if wi % 2 == 0:
                nc.vector.tensor_copy(W16[w][:], st[:])
            else:
                nc.scalar.activation(W16[w][:], st[:], AF.Copy)

        if not zero_bias:
            bqT = const.tile([P, NDB], F32, tag="bqT")
            nc.sync.dma_start(bqT[:], Bd["bq"].ap().rearrange("(o p) -> p o", p=P))
            bkT = const.tile([P, NDB], F32, tag="bkT")
            nc.sync.dma_start(bkT[:], Bd["bk"].ap().rearrange("(o p) -> p o", p=P))
            bc = {}
            for v in ("bv", "bo"):
                st = stage.tile([1, D], F32, tag="vstage")
                nc.sync.dma_start(st[:], Bd[v].ap()[None, :])
                bc[v] = const.tile([P, D], F32, tag=f"bc_{v}", name=f"bc_{v}")
                nc.gpsimd.partition_broadcast(bc[v][:], st[:])
            bv_bc, bo_bc = bc["bv"], bc["bo"]
        if not unit_ln:
            gbc = {}
            for v in ("ln0_g", "ln0_b", "ln1_g", "ln1_b"):
                st = stage.tile([1, D], F32, tag="vstage")
                nc.sync.dma_start(st[:], Bd[v].ap()[None, :])
                gbc[v] = const.tile([P, D], F32, tag=f"bc_{v}", name=f"bc_{v}")
                nc.gpsimd.partition_broadcast(gbc[v][:], st[:])

        # ---- per-batch state ----
        N16 = [{} for _ in range(NB)]       # name -> [128, NBLK, D] bf16
        T16S = [{} for _ in range(NB)]      # name -> [4 tiles d-major]
        PROJ = [{} for _ in range(NB)]      # "qT"/"kT" -> [4 tiles]
        VAUG = [None] * NB
        QNAT = [None] * NB
        AT = [[None] * H for _ in range(NB)]
        SUMS = [None] * NB
        RN = [None] * NB
        OH = [None] * NB
        LN0 = [None] * NB
        LNT = [None] * NB
        PRE2 = [None] * NB

        def emit_load(b, name, halves=False):
            dram = Qd if name == "Q" else Kd
            ld = loadp.tile([P, NBLK, D], F32, tag="ld", name="ld")
            n16 = n16p.tile([P, NBLK, D], BF16, tag="n16", name="n16")
            dview = dram[b].rearrange("(si p) d -> p si d", p=P)
            nh = 2 if halves else 1
            hb = NBLK // nh
            for hf in range(nh):
                sl = slice(hf * hb, (hf + 1) * hb)
                nc.sync.dma_start(ld[:, sl, :], dview[:, sl, :])
                if name == "Q":
                    nc.scalar.activation(n16[:, sl, :], ld[:, sl, :], AF.Copy)
                else:
                    nc.vector.tensor_copy(n16[:, sl, :], ld[:, sl, :])
            N16[b][name] = n16

        def ln_stats_si(src):
            mv = statp.tile([P, 2], F32, tag="mv1", name="mv1", bufs=6)
            st6 = statp.tile([P, 6], F32, tag="st6", name="st6")
            nc.vector.bn_stats(st6[:], src[:])
            nc.vector.bn_aggr(mv[:], st6[:])
            lnv = statp.tile([P, 1], F32, tag="lnv1", name="lnv1", bufs=6)
            nc.scalar.activation(lnv[:], mv[:, 1:2], AF.Ln, bias=epsT[:])
            istd = statp.tile([P, 1], F32, tag="istd1", name="istd1", bufs=6)
            nc.scalar.activation(istd[:], lnv[:], AF.Exp, scale=-0.5)
            return mv, istd

        def ln_apply_si(dst, src, mv, istd, g_bc, b_bc):
            if g_bc is None:
                nc.vector.tensor_scalar(
                    dst, src[:], mv[:, 0:1], istd[:], OP.subtract, OP.mult,
                )
            else:
                t = statp.tile([P, D], F32, tag="lntmp", name="lntmp")
                nc.vector.tensor_scalar(
                    t[:], src[:], mv[:, 0:1], istd[:], OP.subtract, OP.mult,
                )
                t2 = statp.tile([P, D], F32, tag="lntmp2", name="lntmp2")
                nc.vector.tensor_tensor(t2[:], t[:], g_bc[:], OP.mult)
                nc.vector.tensor_tensor(dst, t2[:], b_bc[:], OP.add)

        def ln_stats(srcs):
            """srcs: list of NBLK [128, 512] tiles -> (mv4, istd4)."""
            mv4 = statp.tile([P, NBLK, 2], F32, tag="mv4", name="mv4")
            for si in range(NBLK):
                st6 = statp.tile([P, 6], F32, tag="st6", name="st6")
                nc.vector.bn_stats(st6[:], srcs[si][:])
                nc.vector.bn_aggr(mv4[:, si, :], st6[:])
            lnv = statp.tile([P, NBLK], F32, tag="lnv", name="lnv")
            nc.scalar.activation(lnv[:], mv4[:, :, 1], AF.Ln, bias=epsT[:])
            istd4 = statp.tile([P, NBLK], F32, tag="istd4", name="istd4")
            nc.scalar.activation(istd4[:], lnv[:], AF.Exp, scale=-0.5)
            return mv4, istd4

        def ln_apply_into(dst, src, mv4, istd4, si, g_bc, b_bc):
            if g_bc is None:
                nc.vector.tensor_scalar(
                    dst, src[:], mv4[:, si, 0:1], istd4[:, si:si + 1],
                    OP.subtract, OP.mult,
                )
            else:
                t = statp.tile([P, D], F32, tag="lntmp", name="lntmp")
                nc.vector.tensor_scalar(
                    t[:], src[:], mv4[:, si, 0:1], istd4[:, si:si + 1],
                    OP.subtract, OP.mult,
                )
                t2 = statp.tile([P, D], F32, tag="lntmp2", name="lntmp2")
                nc.vector.tensor_tensor(t2[:], t[:], g_bc[:], OP.mult)
                nc.vector.tensor_tensor(dst, t2[:], b_bc[:], OP.add)

        def emit_A_group(b, g):
            name, dj = ("Q", g) if g < NDB else ("K", g - NDB)
            n16 = N16[b][name]
            ps = mp.tile([P, S], BF16, tag="mp", name="psA")
            for si in range(NBLK):
                nc.tensor.transpose(
                    ps[:, si * P:(si + 1) * P],
                    n16[:, si, dj * P:(dj + 1) * P],
                    I128b[:],
                )
            t16 = t16p.tile([P, S], BF16, tag="t16", name="t16")
            nc.vector.tensor_copy(t16[:], ps[:])
            T16S[b].setdefault(name, []).append(t16)

        def emit_B_group(b, g):
            QT16, KT16 = T16S[b]["Q"], T16S[b]["K"]
            if g < 8:  # qT (g 0-3) / kT (g 4-7)
                wname = "Wq" if g < NDB else "Wk"
                bT = None if zero_bias else (bqT if g < NDB else bkT)
                src = QT16 if g < NDB else KT16
                vi = g % NDB
                ps = mp.tile([P, S], F32, tag="mp", name="psB")
                for dj in range(NDB):
                    nc.tensor.matmul(
                        ps[:],
                        W16[wname][:, dj, vi * P:(vi + 1) * P],
                        src[dj][:],
                        start=(dj == 0),
                        stop=(dj == NDB - 1),
                    )
                t = projp.tile([P, S], BF16, tag="projT", name="projT")
                if g < NDB:
                    # qT drains on ACT (balance), kT on DVE
                    if bT is None:
                        nc.scalar.activation(t[:], ps[:], AF.Copy)
                    else:
                        nc.scalar.activation(t[:], ps[:], AF.Identity,
                                             bias=bT[:, vi:vi + 1])
                else:
                    if bT is None:
                        nc.vector.tensor_copy(t[:], ps[:])
                    else:
                        nc.vector.tensor_scalar(
                            t[:], ps[:], bT[:, vi:vi + 1], None, OP.add, None
                        )
                PROJ[b].setdefault("qT" if g < NDB else "kT", []).append(t)
            else:      # v groups (g 8-11)
                si = g - 8
                ps = mp.tile([P, S], F32, tag="mp", name="psV")
                for dj in range(NDB):
                    nc.tensor.matmul(
                        ps[:],
                        KT16[dj][:, si * P:(si + 1) * P],
                        W16["Wv"][:, dj, :],
                        start=(dj == 0),
                        stop=(dj == NDB - 1),
                    )
                if VAUG[b] is None:
                    VAUG[b] = []
                va = vaugp.tile([P, H, DH + 1], BF16, tag="vaug", name="vaug")
                nc.vector.memset(va[:, :, DH:DH + 1], 1.0)
                if zero_bias:
                    nc.vector.tensor_copy(
                        va[:, :, 0:DH], ps.rearrange("p (h d) -> p h d", h=H)
                    )
                else:
                    nc.vector.tensor_tensor(
                        va[:, :, 0:DH],
                        ps.rearrange("p (h d) -> p h d", h=H),
                        bv_bc.rearrange("p (h d) -> p h d", h=H),
                        OP.add,
                    )
                VAUG[b].append(va)

        def emit_qnat_group(b, si):
            qT16 = PROJ[b]["qT"]
            ps = mp.tile([P, S], BF16, tag="mp", name="psQn")
            for vi in range(NDB):
                nc.tensor.transpose(
                    ps[:, vi * P:(vi + 1) * P],
                    qT16[vi][:, si * P:(si + 1) * P],
                    I128b[:],
                )
            if QNAT[b] is None:
                QNAT[b] = []
            qn = qnatp.tile([P, S], BF16, tag="qnat", name="qnat")
            nc.vector.tensor_copy(qn[:], ps[:])
            QNAT[b].append(qn)

        def emit_C_pair(b, hp):
            # heads (2hp, 2hp+1) share feature block hp; per ki both heads'
            # score matmuls -> one [128, 2, 512] PSUM tile, one exp, then both
            # heads' A@V accumulations for that ki.
            qT16, kT16 = PROJ[b]["qT"], PROJ[b]["kT"]
            if SUMS[b] is None:
                SUMS[b] = dramp.tile([H, S], BF16, tag="sums", name="sums")
            vi = hp
            ea = expp.tile([P, 2, NBLK, S], BF16, tag="expA", name="expA")
            pos = [None, None]
            for ki in range(NBLK):
                ps = scp.tile([P, 2, S], F32, tag="scp", name="scp")
                for u in range(2):
                    hof = u * DH
                    nc.tensor.matmul(
                        ps[:, u, :],
                        kT16[vi][hof:hof + DH, ki * P:(ki + 1) * P],
                        qT16[vi][hof:hof + DH, :],
                        start=True,
                        stop=True,
                    )
                nc.scalar.activation(
                    ea[:, :, ki, :], ps[:], AF.Exp, scale=SC,
                )
                for u in range(2):
                    h = 2 * hp + u
                    if ki == 0:
                        pos[u] = pop.tile([P, S], F32, tag="po", name="po")
                    nc.tensor.matmul(
                        pos[u][0:DH + 1, :],
                        VAUG[b][ki][:, h, :],
                        ea[:, u, ki, :],
                        start=(ki == 0),
                        stop=(ki == NBLK - 1),
                    )
            for u in range(2):
                h = 2 * hp + u
                at = atp.tile([DH + 1, S], BF16, tag="at", name="at")
                nc.vector.tensor_copy(at[:], pos[u][0:DH + 1, :])
                nc.sync.dma_start(SUMS[b][h:h + 1, :], at[DH:DH + 1, :])
                AT[b][h] = at

        def emit_sum_gathers(b):
            sg = rnp.tile([P, NBLK, H], BF16, tag="sg", name="sg", bufs=2)
            for si in range(NBLK):
                nc.sync.dma_start(
                    sg[:, si, :],
                    SUMS[b][:, si * P:(si + 1) * P].rearrange("h s -> s h"),
                )
            rn = rnp.tile([P, NBLK, H], F32, tag="rn", name="rn", bufs=2)
            nc.vector.reciprocal(rn[:], sg[:])
            RN[b] = rn

        def emit_D_group(b, si, last):
            rn = RN[b][:, si, :]
            pa = mp.tile([P, S], BF16, tag="mp", name="psD")
            for h in range(H):
                nc.tensor.transpose(
                    pa[:, h * DH:(h + 1) * DH],
                    AT[b][h][0:DH, si * P:(si + 1) * P],
                    I128b[0:DH, 0:DH],
                )
            if OH[b] is None:
                OH[b] = []
            o = ohp.tile([P, D], BF16, tag="oh", name="oh")
            nc.vector.tensor_tensor(
                o.rearrange("p (h d) -> p h d", h=H),
                pa.rearrange("p (h d) -> p h d", h=H),
                rn[:, :, None].to_broadcast((P, H, DH)),
                OP.mult,
            )
            nc.gpsimd.tensor_tensor(o[:], o[:], QNAT[b][si][:], OP.add)
            OH[b].append(o)

        def emit_E(b):
            g0 = None if unit_ln else gbc["ln0_g"]
            b0 = None if unit_ln else gbc["ln0_b"]
            LN0[b] = []
            if b == NB - 1:
                # per-si chains pipeline the un-overlapped epilogue
                for si in range(NBLK):
                    mv, istd = ln_stats_si(OH[b][si])
                    dst = ln0p.tile([P, D], BF16, tag="ln0", name="ln0")
                    ln_apply_si(dst[:], OH[b][si], mv, istd, g0, b0)
                    LN0[b].append(dst)
                return
            mv4, istd4 = ln_stats(OH[b])
            for si in range(NBLK):
                dst = ln0p.tile([P, D], BF16, tag="ln0", name="ln0")
                ln_apply_into(dst[:], OH[b][si], mv4, istd4, si, g0, b0)
                LN0[b].append(dst)

        def emit_F_lnT(b, vi):
            ps = mp.tile([P, S], BF16, tag="mp", name="psF")
            for si in range(NBLK):
                nc.tensor.transpose(
                    ps[:, si * P:(si + 1) * P],
                    LN0[b][si][:, vi * P:(vi + 1) * P],
                    I128b[:],
                )
            if LNT[b] is None:
                LNT[b] = []
            t = lntp.tile([P, S], BF16, tag="lnT", name="lnT")
            nc.vector.tensor_copy(t[:], ps[:])
            LNT[b].append(t)

        def emit_F_fc(b, si):
            ps = mp.tile([P, S], F32, tag="mp", name="psFc")
            for dj in range(NDB):
                nc.tensor.matmul(
                    ps[:],
                    LNT[b][dj][:, si * P:(si + 1) * P],
                    W16["Wo"][:, dj, :],
                    start=(dj == 0),
                    stop=(dj == NDB - 1),
                )
            if PRE2[b] is None:
                PRE2[b] = []
            p2 = p2p.tile([P, D], BF16, tag="pre2", name="pre2")
            if zero_bias:
                # p2 = relu(fc) + ln0 fused: (ps max 0) + ln0
                nc.vector.scalar_tensor_tensor(
                    p2[:], ps[:], 0.0, LN0[b][si][:], OP.max, OP.add
                )
            else:
                tmp = statp.tile([P, D], F32, tag="fcb", name="fcb")
                nc.vector.tensor_tensor(tmp[:], ps[:], bo_bc[:], OP.add)
                rl = statp.tile([P, D], BF16, tag="relu", name="relu")
                nc.scalar.activation(rl[:], tmp[:], AF.Relu)
                nc.vector.tensor_tensor(p2[:], rl[:], LN0[b][si][:], OP.add)
            PRE2[b].append(p2)

        def emit_E2(b):
            g1 = None if unit_ln else gbc["ln1_g"]
            b1 = None if unit_ln else gbc["ln1_b"]
            if b == NB - 1:
                for si in range(NBLK):
                    mv, istd = ln_stats_si(PRE2[b][si])
                    of1 = outp.tile([P, D], F32, tag="outf1", name="outf1",
                                    bufs=2)
                    ln_apply_si(of1[:], PRE2[b][si], mv, istd, g1, b1)
                    nc.sync.dma_start(Od[b, si * P:(si + 1) * P, :], of1[:])
                return
            mv4b, istd4b = ln_stats(PRE2[b])
            of = outp.tile([P, NBLK, D], F32, tag="outf", name="outf")
            for si in range(NBLK):
                ln_apply_into(of[:, si, :], PRE2[b][si], mv4b, istd4b, si,
                              g1, b1)
            nc.sync.dma_start(
                Od[b].rearrange("(si p) d -> p si d", p=P), of[:]
            )

        # ---- staged emission: 3 batches in flight ----
        def tail_pieces(b):
            last = (b == NB - 1)
            th = []
            for si in range(NBLK):
                th.append(lambda si=si: emit_D_group(b, si, last))
            th.append(lambda: emit_E(b))
            for vi in range(NDB):
                th.append(lambda vi=vi: emit_F_lnT(b, vi))
            for si in range(NBLK):
                th.append(lambda si=si: emit_F_fc(b, si))
            return th  # 13 pieces; E2 emitted separately (post-pairs)

        def prep_pieces(nb):
            th = []
            for g in range(2 * NDB):
                th.append(lambda g=g: emit_A_group(nb, g))
            for g in range(12):
                th.append(lambda g=g: emit_B_group(nb, g))
            return th  # 20 pieces; qnat emitted post-pairs

        # prologue: batch-0 path to first PE work
        emit_load(0, "Q", halves=True)
        emit_weight_load("Wq", 0)
        emit_load(0, "K", halves=True)
        emit_weight_load("Wk", 1)
        emit_weight_load("Wv", 2)
        emit_weight_load("Wo", 3)
        for th in prep_pieces(0):
            th()

        for b in range(NB):
            nb = b + 1
            tails = tail_pieces(b - 1) if b > 0 else []
            if b == 0:
                tails = [lambda si=si: emit_qnat_group(0, si)
                         for si in range(NBLK)]
                tails.append(lambda: emit_load(1, "Q"))
                tails.append(lambda: emit_load(1, "K"))
            preps = prep_pieces(nb) if nb < NB else []
            # A-groups first so their PSUM drains lead the DVE queue (keeps
            # the mp ring moving), then D/E, then B chains, then lnT/fc
            if b == 0:
                # tails here = [qnat(0) x4, load(1) x2]; loads must precede
                # the A(1) groups they feed
                inter = tails[4:] + preps[:8] + tails[:4] + preps[8:]
            else:
                inter = preps[:8] + tails[:5] + preps[8:] + tails[5:]
            n_per = (len(inter) + 3) // 4 if inter else 0
            fi = 0
            for hp in range(H // 2):
                emit_C_pair(b, hp)
                if hp == H // 2 - 1:
                    emit_sum_gathers(b)
                for _ in range(n_per):
                    if fi < len(inter):
                        inter[fi]()
                        fi += 1
            while fi < len(inter):
                inter[fi]()
                fi += 1
            if b > 0:
                emit_E2(b - 1)
            if nb < NB:
                for si in range(NBLK):
                    emit_qnat_group(nb, si)
            if b + 2 < NB:
                emit_load(b + 2, "Q")
                emit_load(b + 2, "K")

        # epilogue: last batch tail, fully serial
        for th in tail_pieces(NB - 1):
            th()
        emit_E2(NB - 1)


_CACHE = {}


def _get_program(zero_bias: bool, unit_ln: bool):
    key = (zero_bias, unit_ln)
    if key not in _CACHE:
        _CACHE[key] = build_program(zero_bias, unit_ln)
    return _CACHE[key]


def _make_in_maps(inputs):
    Q = np.ascontiguousarray(inputs["Q"], dtype=np.float32)
    K = np.ascontiguousarray(inputs["K"], dtype=np.float32)
    shared = {
        name: np.ascontiguousarray(inputs[name], dtype=np.float32)
        for name in ("Wq", "Wk", "Wv", "Wo", "bq", "bk", "bv", "bo",
                     "ln0_g", "ln0_b", "ln1_g", "ln1_b")
    }
    in_maps = []
    for c in range(NCORES):
        m = dict(shared)
        m["Q"] = Q[c * NB:(c + 1) * NB]
        m["K"] = K[c * NB:(c + 1) * NB]
        in_maps.append(m)
    return in_maps


def run(inputs, trace=False):
    zero_bias = all(
        not np.any(inputs[v]) for v in ("bq", "bk", "bv", "bo")
    )
    unit_ln = (
        np.all(inputs["ln0_g"] == 1.0) and np.all(inputs["ln1_g"] == 1.0)
        and not np.any(inputs["ln0_b"]) and not np.any(inputs["ln1_b"])
    )
    nc = _get_program(zero_bias, unit_ln)
    res = run_bass_kernel_spmd(
        nc, _make_in_maps(inputs), core_ids=list(range(NCORES)), trace=trace
    )
    out = np.concatenate([res.results[c]["out"] for c in range(NCORES)], axis=0)
    return out, res


def kernel(**inputs):
    B, Sq, Dq = inputs["Q"].shape
    assert (B, Sq, Dq) == (NB * NCORES, S, D), (B, Sq, Dq)
    out, _ = run(inputs, trace=False)
    return out


# revision 20
# speedup vs baseline: 1.0215x; 1.0215x over previous
"""Trainium2 Bass/Tile kernel for MAB-style attention block (nn_MAB_channel_aware_force).

Reference computation (per batch b of 32):
  q = Q @ Wq + bq ; k = K @ Wk + bk ; v = K @ Wv + bv          # [512, 512]
  per head h (8 heads, dh=64):
    scores = qh @ kh^T / sqrt(512) ; A = softmax(scores)
    oh = qh + A @ vh
  O = LN0(concat(oh)) ; O = O + relu(O @ Wo + bo) ; out = LN1(O)

Sharding: data-parallel over batch across 8 NeuronCores (4 batches/core).

v4 structure per core:
  - attention (C) per head-pair: per ki, both heads' score matmuls land in one
    [128, 2, 512] PSUM tile (adjacent in the PE queue), one exp per ki covers
    both heads, then both heads' A@V accumulations for that ki.  scp is
    double-buffered so scores(ki+1) overlap exp(ki); C is ACT-bound.
  - three batches in flight: C(b) emission interleaves the D/E/F/E2 tail of
    batch b-1 AND the A/B/qnat prep of batch b+1, so the ACT queue never
    head-of-line blocks the next batch's exps behind LN-stats ops.
  - attn^T + softmax sums drained in one DVE copy per head ([65, 512] bf16,
    sums row 64); sums DMA straight from SBUF; relu+residual fused in one
    scalar_tensor_tensor; DMAs coalesced (weights 1/matrix, inputs 2/batch,
    output 1/batch) to relieve the HWDGE issue queue.
"""

import numpy as np

import bass_rust as _bass_rust
import concourse.bass as bass
import concourse.mybir as mybir
import concourse.tile as tile
from concourse import bacc
from concourse.bass_utils import run_bass_kernel_spmd
from concourse.hw_specs import get_activation_tables
from concourse.masks import make_identity


class _BaccOneActTable(bacc.Bacc):
    """Bacc whose act-table pass is pinned to natural_log_exp_and_others.

    The stock greedy pass maps exp -> exp_and_others and ln -> natural_log
    (first set containing each function), forcing ~2.6us of ACT table
    reloads around every LayerNorm rsqrt (ln+exp) and again before the
    next softmax exp.  Every activation this kernel uses (exp, ln, copy,
    identity, relu) lives in the combined natural_log_exp_and_others set,
    so restricting the pass to that set yields exactly one table load.
    Set ids stay aligned with act_info.json (only the contents offered to
    the chooser are masked)."""

    _ACT_SET = "natural_log_exp_and_others"

    def insert_act_table_loads(self):
        has_activation = any(
            isinstance(i, mybir.InstActivation)
            for b in self.main_func.blocks
            for i in b.instructions
        )
        if not has_activation:
            return
        tables = [
            (name, (fns if name == self._ACT_SET else set()))
            for name, fns in get_activation_tables(self.m.arch).items()
        ]
        _bass_rust.insert_act_table_loads(self, tables)

P = 128
S = 512          # sequence length (Sq == Sk)
D = 512          # model dim == DIM_Q == DIM_K == DIM_V
H = 8            # heads
DH = D // H      # 64
NB = 4           # batches per core
NCORES = 8
EPS = 1e-5
SC = 1.0 / float(np.sqrt(D))
F32 = mybir.dt.float32
BF16 = mybir.dt.bfloat16
AF = mybir.ActivationFunctionType
OP = mybir.AluOpType

NBLK = S // P    # 4 sequence blocks of 128
NDB = D // P     # 4 feature blocks of 128


def build_program(zero_bias: bool, unit_ln: bool):
    nc = _BaccOneActTable("TRN2", target_bir_lowering=False, debug=False)

    Qd = nc.declare_dram_parameter("Q", [NB, S, D], F32, isOutput=False)
    Kd = nc.declare_dram_parameter("K", [NB, S, D], F32, isOutput=False)
    Wd = {}
    for w in ("Wq", "Wk", "Wv", "Wo"):
        Wd[w] = nc.declare_dram_parameter(w, [D, D], F32, isOutput=False)
    Bd = {}
    for v in ("bq", "bk", "bv", "bo", "ln0_g", "ln0_b", "ln1_g", "ln1_b"):
        Bd[v] = nc.declare_dram_parameter(v, [D], F32, isOutput=False)
    Od = nc.declare_dram_parameter("out", [NB, S, D], F32, isOutput=True)

    with tile.TileContext(nc) as tc:
        _build(nc, tc, Qd, Kd, Wd, Bd, Od, zero_bias, unit_ln)
    nc.compile()
    return nc


def _build(nc, tc, Qd, Kd, Wd, Bd, Od, zero_bias, unit_ln):
    from contextlib import ExitStack

    ctx = ExitStack()
    with ctx:
        const = ctx.enter_context(tc.tile_pool(name="const", bufs=1))
        stage = ctx.enter_context(tc.tile_pool(name="stage", bufs=2))
        loadp = ctx.enter_context(tc.tile_pool(name="loadp", bufs=4))
        n16p = ctx.enter_context(tc.tile_pool(name="n16p", bufs=5))
        t16p = ctx.enter_context(tc.tile_pool(name="t16p", bufs=12))
        projp = ctx.enter_context(tc.tile_pool(name="projp", bufs=17))
        vaugp = ctx.enter_context(tc.tile_pool(name="vaugp", bufs=9))
        qnatp = ctx.enter_context(tc.tile_pool(name="qnatp", bufs=9))
        expp = ctx.enter_context(tc.tile_pool(name="expp", bufs=2))
        atp = ctx.enter_context(tc.tile_pool(name="atp", bufs=10))
        rnp = ctx.enter_context(tc.tile_pool(name="rnp", bufs=6))
        ohp = ctx.enter_context(tc.tile_pool(name="ohp", bufs=5))
        ln0p = ctx.enter_context(tc.tile_pool(name="ln0p", bufs=5))
        lntp = ctx.enter_context(tc.tile_pool(name="lntp", bufs=5))
        p2p = ctx.enter_context(tc.tile_pool(name="p2p", bufs=5))
        outp = ctx.enter_context(tc.tile_pool(name="outp", bufs=1))
        statp = ctx.enter_context(tc.tile_pool(name="statp", bufs=10))

        dramp = ctx.enter_context(tc.tile_pool(name="dramp", bufs=3, space="DRAM"))
        # PSUM (8 banks): scores pairs 2x[2 banks], attn-out 2x[1], misc
        # (proj/fc/transposes) 2x[1].
        scp = ctx.enter_context(tc.tile_pool(name="scp", bufs=2, space="PSUM"))
        pop = ctx.enter_context(tc.tile_pool(name="pop", bufs=2, space="PSUM"))
        mp = ctx.enter_context(tc.tile_pool(name="mp", bufs=2, space="PSUM"))

        # ---- one-time constants ----
        I128b = const.tile([P, P], BF16)
        make_identity(nc, I128b)
        epsT = const.tile([P, 1], F32)
        nc.vector.memset(epsT[:], EPS)

        W16 = {}

        def emit_weight_load(w, wi):
            W16[w] = const.tile([P, NDB, D], BF16, tag=f"w16_{w}", name=f"w16_{w}")
            st = loadp.tile([P, NDB, D], F32, tag="wld", name="wld", bufs=2)
            nc.sync.dma_start(st[:], Wd[w].ap().rearrange("(o p) n -> p o n", p=P))
            if wi % 2 == 0:
                nc.vector.tensor_copy(W16[w][:], st[:])
            else:
                nc.scalar.activation(W16[w][:], st[:], AF.Copy)

        if not zero_bias:
            bqT = const.tile([P, NDB], F32, tag="bqT")
            nc.sync.dma_start(bqT[:], Bd["bq"].ap().rearrange("(o p) -> p o", p=P))
            bkT = const.tile([P, NDB], F32, tag="bkT")
            nc.sync.dma_start(bkT[:], Bd["bk"].ap().rearrange("(o p) -> p o", p=P))
            bc = {}
            for v in ("bv", "bo"):
                st = stage.tile([1, D], F32, tag="vstage")
                nc.sync.dma_start(st[:], Bd[v].ap()[None, :])
                bc[v] = const.tile([P, D], F32, tag=f"bc_{v}", name=f"bc_{v}")
                nc.gpsimd.partition_broadcast(bc[v][:], st[:])
            bv_bc, bo_bc = bc["bv"], bc["bo"]
        if not unit_ln:
            gbc = {}
            for v in ("ln0_g", "ln0_b", "ln1_g", "ln1_b"):
                st = stage.tile([1, D], F32, tag="vstage")
                nc.sync.dma_start(st[:], Bd[v].ap()[None, :])
                gbc[v] = const.tile([P, D], F32, tag=f"bc_{v}", name=f"bc_{v}")
                nc.gpsimd.partition_broadcast(gbc[v][:], st[:])

        # ---- per-batch state ----
        N16 = [{} for _ in range(NB)]       # name -> [128, NBLK, D] bf16
        T16S = [{} for _ in range(NB)]      # name -> [4 tiles d-major]
        PROJ = [{} for _ in range(NB)]      # "qT"/"kT" -> [4 tiles]
        VAUG = [None] * NB
        QNAT = [None] * NB
        AT = [[None] * H for _ in range(NB)]
        SUMS = [None] * NB
        RN = [None] * NB
        OH = [None] * NB
        LN0 = [None] * NB
        LNT = [None] * NB
        PRE2 = [None] * NB

        def emit_load(b, name, halves=False):
            dram = Qd if name == "Q" else Kd
            ld = loadp.tile([P, NBLK, D], F32, tag="ld", name="ld")
            n16 = n16p.tile([P, NBLK, D], BF16, tag="n16", name="n16")
            dview = dram[b].rearrange("(si p) d -> p si d", p=P)
            nh = 2 if halves else 1
            hb = NBLK // nh
            for hf in range(nh):
                sl = slice(hf * hb, (hf + 1) * hb)
                nc.sync.dma_start(ld[:, sl, :], dview[:, sl, :])
                if name == "Q":
                    nc.scalar.activation(n16[:, sl, :], ld[:, sl, :], AF.Copy)
                else:
                    nc.vector.tensor_copy(n16[:, sl, :], ld[:, sl, :])
            N16[b][name] = n16

        def ln_stats_si(src):
            mv = statp.tile([P, 2], F32, tag="mv1", name="mv1", bufs=6)
            st6 = statp.tile([P, 6], F32, tag="st6", name="st6")
            nc.vector.bn_stats(st6[:], src[:])
            nc.vector.bn_aggr(mv[:], st6[:])
            lnv = statp.tile([P, 1], F32, tag="lnv1", name="lnv1", bufs=6)
            nc.scalar.activation(lnv[:], mv[:, 1:2], AF.Ln, bias=epsT[:])
            istd = statp.tile([P, 1], F32, tag="istd1", name="istd1", bufs=6)
            nc.scalar.activation(istd[:], lnv[:], AF.Exp, scale=-0.5)
            return mv, istd

        def ln_apply_si(dst, src, mv, istd, g_bc, b_bc):
            if g_bc is None:
                nc.vector.tensor_scalar(
                    dst, src[:], mv[:, 0:1], istd[:], OP.subtract, OP.mult,
                )
            else:
                t = statp.tile([P, D], F32, tag="lntmp", name="lntmp")
                nc.vector.tensor_scalar(
                    t[:], src[:], mv[:, 0:1], istd[:], OP.subtract, OP.mult,
                )
                t2 = statp.tile([P, D], F32, tag="lntmp2", name="lntmp2")
                nc.vector.tensor_tensor(t2[:], t[:], g_bc[:], OP.mult)
                nc.vector.tensor_tensor(dst, t2[:], b_bc[:], OP.add)

        def ln_stats(srcs):
            """srcs: list of NBLK [128, 512] tiles -> (mv4, istd4)."""
            mv4 = statp.tile([P, NBLK, 2], F32, tag="mv4", name="mv4")
            for si in range(NBLK):
                st6 = statp.tile([P, 6], F32, tag="st6", name="st6")
                nc.vector.bn_stats(st6[:], srcs[si][:])
                nc.vector.bn_aggr(mv4[:, si, :], st6[:])
            lnv = statp.tile([P, NBLK], F32, tag="lnv", name="lnv")
            nc.scalar.activation(lnv[:], mv4[:, :, 1], AF.Ln, bias=epsT[:])
            istd4 = statp.tile([P, NBLK], F32, tag="istd4", name="istd4")
            nc.scalar.activation(istd4[:], lnv[:], AF.Exp, scale=-0.5)
            return mv4, istd4

        def ln_apply_into(dst, src, mv4, istd4, si, g_bc, b_bc):
            if g_bc is None:
                nc.vector.tensor_scalar(
                    dst, src[:], mv4[:, si, 0:1], istd4[:, si:si + 1],
                    OP.subtract, OP.mult,
                )
            else:
                t = statp.tile([P, D], F32, tag="lntmp", name="lntmp")
                nc.vector.tensor_scalar(
                    t[:], src[:], mv4[:, si, 0:1], istd4[:, si:si + 1],
                    OP.subtract, OP.mult,
                )
                t2 = statp.tile([P, D], F32, tag="lntmp2", name="lntmp2")
                nc.vector.tensor_tensor(t2[:], t[:], g_bc[:], OP.mult)
                nc.vector.tensor_tensor(dst, t2[:], b_bc[:], OP.add)

        def emit_A_group(b, g):
            name, dj = ("Q", g) if g < NDB else ("K", g - NDB)
            n16 = N16[b][name]
            ps = mp.tile([P, S], BF16, tag="mp", name="psA")
            for si in range(NBLK):
                nc.tensor.transpose(
                    ps[:, si * P:(si + 1) * P],
                    n16[:, si, dj * P:(dj + 1) * P],
                    I128b[:],
                )
            t16 = t16p.tile([P, S], BF16, tag="t16", name="t16")
            nc.vector.tensor_copy(t16[:], ps[:])
            T16S[b].setdefault(name, []).append(t16)

        def emit_B_group(b, g):
            QT16, KT16 = T16S[b]["Q"], T16S[b]["K"]
            if g < 8:  # qT (g 0-3) / kT (g 4-7)
                wname = "Wq" if g < NDB else "Wk"
                bT = None if zero_bias else (bqT if g < NDB else bkT)
                src = QT16 if g < NDB else KT16
                vi = g % NDB
                ps = mp.tile([P, S], F32, tag="mp", name="psB")
                for dj in range(NDB):
                    nc.tensor.matmul(
                        ps[:],
                        W16[wname][:, dj, vi * P:(vi + 1) * P],
                        src[dj][:],
                        start=(dj == 0),
                        stop=(dj == NDB - 1),
                    )
                t = projp.tile([P, S], BF16, tag="projT", name="projT")
                if g < NDB:
                    # qT drains on ACT (balance), kT on DVE
                    if bT is None:
                        nc.scalar.activation(t[:], ps[:], AF.Copy)
                    else:
                        nc.scalar.activation(t[:], ps[:], AF.Identity,
                                             bias=bT[:, vi:vi + 1])
                else:
                    if bT is None:
                        nc.vector.tensor_copy(t[:], ps[:])
                    else:
                        nc.vector.tensor_scalar(
                            t[:], ps[:], bT[:, vi:vi + 1], None, OP.add, None
                        )
                PROJ[b].setdefault("qT" if g < NDB else "kT", []).append(t)
            else:      # v groups (g 8-11)
                si = g - 8
                ps = mp.tile([P, S], F32, tag="mp", name="psV")
                for dj in range(NDB):
                    nc.tensor.matmul(
                        ps[:],
                        KT16[dj][:, si * P:(si + 1) * P],
                        W16["Wv"][:, dj, :],
                        start=(dj == 0),
                        stop=(dj == NDB - 1),
                    )
                if VAUG[b] is None:
                    VAUG[b] = []
                va = vaugp.tile([P, H, DH + 1], BF16, tag="vaug", name="vaug")
                nc.vector.memset(va[:, :, DH:DH + 1], 1.0)
                if zero_bias:
                    nc.vector.tensor_copy(
                        va[:, :, 0:DH], ps.rearrange("p (h d) -> p h d", h=H)
                    )
                else:
                    nc.vector.tensor_tensor(
                        va[:, :, 0:DH],
                        ps.rearrange("p (h d) -> p h d", h=H),
                        bv_bc.rearrange("p (h d) -> p h d", h=H),
                        OP.add,
                    )
                VAUG[b].append(va)

        def emit_qnat_group(b, si):
            qT16 = PROJ[b]["qT"]
            ps = mp.tile([P, S], BF16, tag="mp", name="psQn")
            for vi in range(NDB):
                nc.tensor.transpose(
                    ps[:, vi * P:(vi + 1) * P],
                    qT16[vi][:, si * P:(si + 1) * P],
                    I128b[:],
                )
            if QNAT[b] is None:
                QNAT[b] = []
            qn = qnatp.tile([P, S], BF16, tag="qnat", name="qnat")
            nc.vector.tensor_copy(qn[:], ps[:])
            QNAT[b].append(qn)

        def emit_C_pair(b, hp):
            # heads (2hp, 2hp+1) share feature block hp; per ki both heads'
            # score matmuls -> one [128, 2, 512] PSUM tile, one exp, then both
            # heads' A@V accumulations for that ki.
            qT16, kT16 = PROJ[b]["qT"], PROJ[b]["kT"]
            if SUMS[b] is None:
                SUMS[b] = dramp.tile([H, S], BF16, tag="sums", name="sums")
            vi = hp
            ea = expp.tile([P, 2, NBLK, S], BF16, tag="expA", name="expA")
            pos = [None, None]
            for ki in range(NBLK):
                ps = scp.tile([P, 2, S], F32, tag="scp", name="scp")
                for u in range(2):
                    hof = u * DH
                    nc.tensor.matmul(
                        ps[:, u, :],
                        kT16[vi][hof:hof + DH, ki * P:(ki + 1) * P],
                        qT16[vi][hof:hof + DH, :],
                        start=True,
                        stop=True,
                    )
                nc.scalar.activation(
                    ea[:, :, ki, :], ps[:], AF.Exp, scale=SC,
                )
                for u in range(2):
                    h = 2 * hp + u
                    if ki == 0:
                        pos[u] = pop.tile([P, S], F32, tag="po", name="po")
                    nc.tensor.matmul(
                        pos[u][0:DH + 1, :],
                        VAUG[b][ki][:, h, :],
                        ea[:, u, ki, :],
                        start=(ki == 0),
                        stop=(ki == NBLK - 1),
                    )
            for u in range(2):
                h = 2 * hp + u
                at = atp.tile([DH + 1, S], BF16, tag="at", name="at")
                nc.vector.tensor_copy(at[:], pos[u][0:DH + 1, :])
                nc.sync.dma_start(SUMS[b][h:h + 1, :], at[DH:DH + 1, :])
                AT[b][h] = at

        def emit_sum_gathers(b):
            RN[b] = []
            for si in range(NBLK):
                sg = rnp.tile([P, H], BF16, tag="sg", name="sg", bufs=5)
                nc.sync.dma_start(
                    sg[:],
                    SUMS[b][:, si * P:(si + 1) * P].rearrange("h s -> s h"),
                )
                rn = rnp.tile([P, H], F32, tag="rn", name="rn", bufs=8)
                nc.vector.reciprocal(rn[:], sg[:])
                RN[b].append(rn)

        def emit_D_group(b, si, last):
            rn = RN[b][si]
            pa = mp.tile([P, S], BF16, tag="mp", name="psD")
            for h in range(H):
                nc.tensor.transpose(
                    pa[:, h * DH:(h + 1) * DH],
                    AT[b][h][0:DH, si * P:(si + 1) * P],
                    I128b[0:DH, 0:DH],
                )
            if OH[b] is None:
                OH[b] = []
            o = ohp.tile([P, D], BF16, tag="oh", name="oh")
            nc.vector.tensor_tensor(
                o.rearrange("p (h d) -> p h d", h=H),
                pa.rearrange("p (h d) -> p h d", h=H),
                rn[:, :, None].to_broadcast((P, H, DH)),
                OP.mult,
            )
            if last:
                nc.vector.tensor_tensor(o[:], o[:], QNAT[b][si][:], OP.add)
            else:
                nc.gpsimd.tensor_tensor(o[:], o[:], QNAT[b][si][:], OP.add)
            OH[b].append(o)

        def emit_E(b):
            g0 = None if unit_ln else gbc["ln0_g"]
            b0 = None if unit_ln else gbc["ln0_b"]
            LN0[b] = []
            mv4, istd4 = ln_stats(OH[b])
            for si in range(NBLK):
                dst = ln0p.tile([P, D], BF16, tag="ln0", name="ln0")
                ln_apply_into(dst[:], OH[b][si], mv4, istd4, si, g0, b0)
                LN0[b].append(dst)

        def emit_F_lnT(b, vi):
            ps = mp.tile([P, S], BF16, tag="mp", name="psF")
            for si in range(NBLK):
                nc.tensor.transpose(
                    ps[:, si * P:(si + 1) * P],
                    LN0[b][si][:, vi * P:(vi + 1) * P],
                    I128b[:],
                )
            if LNT[b] is None:
                LNT[b] = []
            t = lntp.tile([P, S], BF16, tag="lnT", name="lnT")
            nc.vector.tensor_copy(t[:], ps[:])
            LNT[b].append(t)

        def emit_F_fc(b, si):
            ps = mp.tile([P, S], F32, tag="mp", name="psFc")
            for dj in range(NDB):
                nc.tensor.matmul(
                    ps[:],
                    LNT[b][dj][:, si * P:(si + 1) * P],
                    W16["Wo"][:, dj, :],
                    start=(dj == 0),
                    stop=(dj == NDB - 1),
                )
            if PRE2[b] is None:
                PRE2[b] = []
            p2 = p2p.tile([P, D], BF16, tag="pre2", name="pre2")
            if zero_bias:
                # p2 = relu(fc) + ln0 fused: (ps max 0) + ln0
                nc.vector.scalar_tensor_tensor(
                    p2[:], ps[:], 0.0, LN0[b][si][:], OP.max, OP.add
                )
            else:
                tmp = statp.tile([P, D], F32, tag="fcb", name="fcb")
                nc.vector.tensor_tensor(tmp[:], ps[:], bo_bc[:], OP.add)
                rl = statp.tile([P, D], BF16, tag="relu", name="relu")
                nc.scalar.activation(rl[:], tmp[:], AF.Relu)
                nc.vector.tensor_tensor(p2[:], rl[:], LN0[b][si][:], OP.add)
            PRE2[b].append(p2)

        def emit_E2(b):
            g1 = None if unit_ln else gbc["ln1_g"]
            b1 = None if unit_ln else gbc["ln1_b"]
            mv4b, istd4b = ln_stats(PRE2[b])
            of = outp.tile([P, NBLK, D], F32, tag="outf", name="outf")
            for si in range(NBLK):
                ln_apply_into(of[:, si, :], PRE2[b][si], mv4b, istd4b, si,
                              g1, b1)
            nc.sync.dma_start(
                Od[b].rearrange("(si p) d -> p si d", p=P), of[:]
            )

        # ---- staged emission: 3 batches in flight ----
        def tail_pieces(b):
            last = (b == NB - 1)
            th = []
            for si in range(NBLK):
                th.append(lambda si=si: emit_D_group(b, si, last))
            th.append(lambda: emit_E(b))
            for vi in range(NDB):
                th.append(lambda vi=vi: emit_F_lnT(b, vi))
            for si in range(NBLK):
                th.append(lambda si=si: emit_F_fc(b, si))
            return th  # 13 pieces; E2 emitted separately (post-pairs)

        def prep_pieces(nb):
            th = []
            for g in range(2 * NDB):
                th.append(lambda g=g: emit_A_group(nb, g))
            for g in range(12):
                th.append(lambda g=g: emit_B_group(nb, g))
            return th  # 20 pieces; qnat emitted post-pairs

        # prologue: batch-0 path to first PE work
        emit_load(0, "Q")
        emit_weight_load("Wq", 0)
        emit_load(0, "K")
        emit_weight_load("Wk", 1)
        emit_weight_load("Wv", 2)
        emit_weight_load("Wo", 3)
        for th in prep_pieces(0):
            th()
        for si in range(NBLK):
            emit_qnat_group(0, si)
        emit_load(1, "Q")
        emit_load(1, "K")

        for b in range(NB):
            nb = b + 1
            tails = tail_pieces(b - 1) if b > 0 else []
            preps = prep_pieces(nb) if nb < NB else []
            # A-groups first so their PSUM drains lead the DVE queue (keeps
            # the mp ring moving), then D/E, then B chains, then lnT/fc
            inter = preps[:8] + tails[:5] + preps[8:] + tails[5:]
            n_per = (len(inter) + 3) // 4 if inter else 0
            fi = 0
            for hp in range(H // 2):
                emit_C_pair(b, hp)
                if hp == H // 2 - 1:
                    emit_sum_gathers(b)
                for _ in range(n_per):
                    if fi < len(inter):
                        inter[fi]()
                        fi += 1
            while fi < len(inter):
                inter[fi]()
                fi += 1
            if b > 0:
                emit_E2(b - 1)
            if nb < NB:
                for si in range(NBLK):
                    emit_qnat_group(nb, si)
            if b + 2 < NB:
                emit_load(b + 2, "Q")
                emit_load(b + 2, "K")

        # epilogue: last batch tail, fully serial
        for th in tail_pieces(NB - 1):
            th()
        emit_E2(NB - 1)


_CACHE = {}


def _get_program(zero_bias: bool, unit_ln: bool):
    key = (zero_bias, unit_ln)
    if key not in _CACHE:
        _CACHE[key] = build_program(zero_bias, unit_ln)
    return _CACHE[key]


def _make_in_maps(inputs):
    Q = np.ascontiguousarray(inputs["Q"], dtype=np.float32)
    K = np.ascontiguousarray(inputs["K"], dtype=np.float32)
    shared = {
        name: np.ascontiguousarray(inputs[name], dtype=np.float32)
        for name in ("Wq", "Wk", "Wv", "Wo", "bq", "bk", "bv", "bo",
                     "ln0_g", "ln0_b", "ln1_g", "ln1_b")
    }
    in_maps = []
    for c in range(NCORES):
        m = dict(shared)
        m["Q"] = Q[c * NB:(c + 1) * NB]
        m["K"] = K[c * NB:(c + 1) * NB]
        in_maps.append(m)
    return in_maps


def run(inputs, trace=False):
    zero_bias = all(
        not np.any(inputs[v]) for v in ("bq", "bk", "bv", "bo")
    )
    unit_ln = (
        np.all(inputs["ln0_g"] == 1.0) and np.all(inputs["ln1_g"] == 1.0)
        and not np.any(inputs["ln0_b"]) and not np.any(inputs["ln1_b"])
    )
    nc = _get_program(zero_bias, unit_ln)
    res = run_bass_kernel_spmd(
        nc, _make_in_maps(inputs), core_ids=list(range(NCORES)), trace=trace
    )
    out = np.concatenate([res.results[c]["out"] for c in range(NCORES)], axis=0)
    return out, res


def kernel(**inputs):
    B, Sq, Dq = inputs["Q"].shape
    assert (B, Sq, Dq) == (NB * NCORES, S, D), (B, Sq, Dq)
    out, _ = run(inputs, trace=False)
    return out


# revision 21
# speedup vs baseline: 1.0578x; 1.0355x over previous
"""Trainium2 Bass/Tile kernel for MAB-style attention block (nn_MAB_channel_aware_force).

Reference computation (per batch b of 32):
  q = Q @ Wq + bq ; k = K @ Wk + bk ; v = K @ Wv + bv          # [512, 512]
  per head h (8 heads, dh=64):
    scores = qh @ kh^T / sqrt(512) ; A = softmax(scores)
    oh = qh + A @ vh
  O = LN0(concat(oh)) ; O = O + relu(O @ Wo + bo) ; out = LN1(O)

Sharding: data-parallel over batch across 8 NeuronCores (4 batches/core).

v4 structure per core:
  - attention (C) per head-pair: per ki, both heads' score matmuls land in one
    [128, 2, 512] PSUM tile (adjacent in the PE queue), one exp per ki covers
    both heads, then both heads' A@V accumulations for that ki.  scp is
    double-buffered so scores(ki+1) overlap exp(ki); C is ACT-bound.
  - three batches in flight: C(b) emission interleaves the D/E/F/E2 tail of
    batch b-1 AND the A/B/qnat prep of batch b+1, so the ACT queue never
    head-of-line blocks the next batch's exps behind LN-stats ops.
  - attn^T + softmax sums drained in one DVE copy per head ([65, 512] bf16,
    sums row 64); sums DMA straight from SBUF; relu+residual fused in one
    scalar_tensor_tensor; DMAs coalesced (weights 1/matrix, inputs 2/batch,
    output 1/batch) to relieve the HWDGE issue queue.
"""

import numpy as np

import bass_rust as _bass_rust
import concourse.bass as bass
import concourse.mybir as mybir
import concourse.tile as tile
from concourse import bacc
from concourse.bass_utils import run_bass_kernel_spmd
from concourse.hw_specs import get_activation_tables
from concourse.masks import make_identity


class _BaccOneActTable(bacc.Bacc):
    """Bacc whose act-table pass is pinned to natural_log_exp_and_others.

    The stock greedy pass maps exp -> exp_and_others and ln -> natural_log
    (first set containing each function), forcing ~2.6us of ACT table
    reloads around every LayerNorm rsqrt (ln+exp) and again before the
    next softmax exp.  Every activation this kernel uses (exp, ln, copy,
    identity, relu) lives in the combined natural_log_exp_and_others set,
    so restricting the pass to that set yields exactly one table load.
    Set ids stay aligned with act_info.json (only the contents offered to
    the chooser are masked)."""

    _ACT_SET = "natural_log_exp_and_others"

    def insert_act_table_loads(self):
        has_activation = any(
            isinstance(i, mybir.InstActivation)
            for b in self.main_func.blocks
            for i in b.instructions
        )
        if not has_activation:
            return
        tables = [
            (name, (fns if name == self._ACT_SET else set()))
            for name, fns in get_activation_tables(self.m.arch).items()
        ]
        _bass_rust.insert_act_table_loads(self, tables)

P = 128
S = 512          # sequence length (Sq == Sk)
D = 512          # model dim == DIM_Q == DIM_K == DIM_V
H = 8            # heads
DH = D // H      # 64
NB = 4           # batches per core
NCORES = 8
EPS = 1e-5
SC = 1.0 / float(np.sqrt(D))
F32 = mybir.dt.float32
BF16 = mybir.dt.bfloat16
AF = mybir.ActivationFunctionType
OP = mybir.AluOpType

NBLK = S // P    # 4 sequence blocks of 128
NDB = D // P     # 4 feature blocks of 128


def build_program(zero_bias: bool, unit_ln: bool):
    nc = _BaccOneActTable("TRN2", target_bir_lowering=False, debug=False)

    Qd = nc.declare_dram_parameter("Q", [NB, S, D], F32, isOutput=False)
    Kd = nc.declare_dram_parameter("K", [NB, S, D], F32, isOutput=False)
    Wd = {}
    for w in ("Wq", "Wk", "Wv", "Wo"):
        Wd[w] = nc.declare_dram_parameter(w, [D, D], F32, isOutput=False)
    Bd = {}
    for v in ("bq", "bk", "bv", "bo", "ln0_g", "ln0_b", "ln1_g", "ln1_b"):
        Bd[v] = nc.declare_dram_parameter(v, [D], F32, isOutput=False)
    Od = nc.declare_dram_parameter("out", [NB, S, D], F32, isOutput=True)

    with tile.TileContext(nc) as tc:
        _build(nc, tc, Qd, Kd, Wd, Bd, Od, zero_bias, unit_ln)
    nc.compile()
    return nc


def _build(nc, tc, Qd, Kd, Wd, Bd, Od, zero_bias, unit_ln):
    from contextlib import ExitStack

    ctx = ExitStack()
    with ctx:
        const = ctx.enter_context(tc.tile_pool(name="const", bufs=1))
        stage = ctx.enter_context(tc.tile_pool(name="stage", bufs=2))
        loadp = ctx.enter_context(tc.tile_pool(name="loadp", bufs=4))
        n16p = ctx.enter_context(tc.tile_pool(name="n16p", bufs=5))
        t16p = ctx.enter_context(tc.tile_pool(name="t16p", bufs=12))
        projp = ctx.enter_context(tc.tile_pool(name="projp", bufs=17))
        vaugp = ctx.enter_context(tc.tile_pool(name="vaugp", bufs=9))
        qnatp = ctx.enter_context(tc.tile_pool(name="qnatp", bufs=9))
        expp = ctx.enter_context(tc.tile_pool(name="expp", bufs=2))
        atp = ctx.enter_context(tc.tile_pool(name="atp", bufs=10))
        rnp = ctx.enter_context(tc.tile_pool(name="rnp", bufs=6))
        ohp = ctx.enter_context(tc.tile_pool(name="ohp", bufs=5))
        ln0p = ctx.enter_context(tc.tile_pool(name="ln0p", bufs=5))
        lntp = ctx.enter_context(tc.tile_pool(name="lntp", bufs=5))
        p2p = ctx.enter_context(tc.tile_pool(name="p2p", bufs=5))
        outp = ctx.enter_context(tc.tile_pool(name="outp", bufs=1))
        statp = ctx.enter_context(tc.tile_pool(name="statp", bufs=10))

        dramp = ctx.enter_context(tc.tile_pool(name="dramp", bufs=3, space="DRAM"))
        # PSUM (8 banks): scores pairs 2x[2 banks], attn-out 2x[1], misc
        # (proj/fc/transposes) 2x[1].
        scp = ctx.enter_context(tc.tile_pool(name="scp", bufs=2, space="PSUM"))
        pop = ctx.enter_context(tc.tile_pool(name="pop", bufs=2, space="PSUM"))
        mp = ctx.enter_context(tc.tile_pool(name="mp", bufs=2, space="PSUM"))

        # ---- one-time constants ----
        I128b = const.tile([P, P], BF16)
        make_identity(nc, I128b)
        epsT = const.tile([P, 1], F32)
        nc.vector.memset(epsT[:], EPS)

        W16 = {}

        def emit_weight_load(w, wi):
            W16[w] = const.tile([P, NDB, D], BF16, tag=f"w16_{w}", name=f"w16_{w}")
            st = loadp.tile([P, NDB, D], F32, tag="wld", name="wld", bufs=2)
            nc.sync.dma_start(st[:], Wd[w].ap().rearrange("(o p) n -> p o n", p=P))
            if wi % 2 == 0:
                nc.vector.tensor_copy(W16[w][:], st[:])
            else:
                nc.scalar.activation(W16[w][:], st[:], AF.Copy)

        if not zero_bias:
            bqT = const.tile([P, NDB], F32, tag="bqT")
            nc.sync.dma_start(bqT[:], Bd["bq"].ap().rearrange("(o p) -> p o", p=P))
            bkT = const.tile([P, NDB], F32, tag="bkT")
            nc.sync.dma_start(bkT[:], Bd["bk"].ap().rearrange("(o p) -> p o", p=P))
            bc = {}
            for v in ("bv", "bo"):
                st = stage.tile([1, D], F32, tag="vstage")
                nc.sync.dma_start(st[:], Bd[v].ap()[None, :])
                bc[v] = const.tile([P, D], F32, tag=f"bc_{v}", name=f"bc_{v}")
                nc.gpsimd.partition_broadcast(bc[v][:], st[:])
            bv_bc, bo_bc = bc["bv"], bc["bo"]
        if not unit_ln:
            gbc = {}
            for v in ("ln0_g", "ln0_b", "ln1_g", "ln1_b"):
                st = stage.tile([1, D], F32, tag="vstage")
                nc.sync.dma_start(st[:], Bd[v].ap()[None, :])
                gbc[v] = const.tile([P, D], F32, tag=f"bc_{v}", name=f"bc_{v}")
                nc.gpsimd.partition_broadcast(gbc[v][:], st[:])

        # ---- per-batch state ----
        N16 = [{} for _ in range(NB)]       # name -> [128, NBLK, D] bf16
        T16S = [{} for _ in range(NB)]      # name -> [4 tiles d-major]
        PROJ = [{} for _ in range(NB)]      # "qT"/"kT" -> [4 tiles]
        VAUG = [None] * NB
        QNAT = [None] * NB
        AT = [[None] * H for _ in range(NB)]
        SUMS = [None] * NB
        RN = [None] * NB
        OH = [None] * NB
        LN0 = [None] * NB
        LNT = [None] * NB
        PRE2 = [None] * NB

        def emit_load(b, name, halves=False):
            dram = Qd if name == "Q" else Kd
            ld = loadp.tile([P, NBLK, D], F32, tag="ld", name="ld")
            n16 = n16p.tile([P, NBLK, D], BF16, tag="n16", name="n16")
            dview = dram[b].rearrange("(si p) d -> p si d", p=P)
            nh = 2 if halves else 1
            hb = NBLK // nh
            for hf in range(nh):
                sl = slice(hf * hb, (hf + 1) * hb)
                nc.sync.dma_start(ld[:, sl, :], dview[:, sl, :])
                if name == "Q":
                    nc.scalar.activation(n16[:, sl, :], ld[:, sl, :], AF.Copy)
                else:
                    nc.vector.tensor_copy(n16[:, sl, :], ld[:, sl, :])
            N16[b][name] = n16

        def ln_stats_si(src):
            mv = statp.tile([P, 2], F32, tag="mv1", name="mv1", bufs=6)
            st6 = statp.tile([P, 6], F32, tag="st6", name="st6")
            nc.vector.bn_stats(st6[:], src[:])
            nc.vector.bn_aggr(mv[:], st6[:])
            lnv = statp.tile([P, 1], F32, tag="lnv1", name="lnv1", bufs=6)
            nc.scalar.activation(lnv[:], mv[:, 1:2], AF.Ln, bias=epsT[:])
            istd = statp.tile([P, 1], F32, tag="istd1", name="istd1", bufs=6)
            nc.scalar.activation(istd[:], lnv[:], AF.Exp, scale=-0.5)
            return mv, istd

        def ln_apply_si(dst, src, mv, istd, g_bc, b_bc):
            if g_bc is None:
                nc.vector.tensor_scalar(
                    dst, src[:], mv[:, 0:1], istd[:], OP.subtract, OP.mult,
                )
            else:
                t = statp.tile([P, D], F32, tag="lntmp", name="lntmp")
                nc.vector.tensor_scalar(
                    t[:], src[:], mv[:, 0:1], istd[:], OP.subtract, OP.mult,
                )
                t2 = statp.tile([P, D], F32, tag="lntmp2", name="lntmp2")
                nc.vector.tensor_tensor(t2[:], t[:], g_bc[:], OP.mult)
                nc.vector.tensor_tensor(dst, t2[:], b_bc[:], OP.add)

        def ln_stats(srcs):
            """srcs: list of NBLK [128, 512] tiles -> (mv4, istd4)."""
            mv4 = statp.tile([P, NBLK, 2], F32, tag="mv4", name="mv4")
            for si in range(NBLK):
                st6 = statp.tile([P, 6], F32, tag="st6", name="st6")
                nc.vector.bn_stats(st6[:], srcs[si][:])
                nc.vector.bn_aggr(mv4[:, si, :], st6[:])
            lnv = statp.tile([P, NBLK], F32, tag="lnv", name="lnv")
            nc.scalar.activation(lnv[:], mv4[:, :, 1], AF.Ln, bias=epsT[:])
            istd4 = statp.tile([P, NBLK], F32, tag="istd4", name="istd4")
            nc.scalar.activation(istd4[:], lnv[:], AF.Exp, scale=-0.5)
            return mv4, istd4

        def ln_apply_into(dst, src, mv4, istd4, si, g_bc, b_bc):
            if g_bc is None:
                nc.vector.tensor_scalar(
                    dst, src[:], mv4[:, si, 0:1], istd4[:, si:si + 1],
                    OP.subtract, OP.mult,
                )
            else:
                t = statp.tile([P, D], F32, tag="lntmp", name="lntmp")
                nc.vector.tensor_scalar(
                    t[:], src[:], mv4[:, si, 0:1], istd4[:, si:si + 1],
                    OP.subtract, OP.mult,
                )
                t2 = statp.tile([P, D], F32, tag="lntmp2", name="lntmp2")
                nc.vector.tensor_tensor(t2[:], t[:], g_bc[:], OP.mult)
                nc.vector.tensor_tensor(dst, t2[:], b_bc[:], OP.add)

        def emit_A_group(b, g):
            name, dj = ("Q", g) if g < NDB else ("K", g - NDB)
            n16 = N16[b][name]
            ps = mp.tile([P, S], BF16, tag="mp", name="psA")
            for si in range(NBLK):
                nc.tensor.transpose(
                    ps[:, si * P:(si + 1) * P],
                    n16[:, si, dj * P:(dj + 1) * P],
                    I128b[:],
                )
            t16 = t16p.tile([P, S], BF16, tag="t16", name="t16")
            nc.vector.tensor_copy(t16[:], ps[:])
            T16S[b].setdefault(name, []).append(t16)

        def emit_B_group(b, g):
            QT16, KT16 = T16S[b]["Q"], T16S[b]["K"]
            if g < 8:  # qT (g 0-3) / kT (g 4-7)
                wname = "Wq" if g < NDB else "Wk"
                bT = None if zero_bias else (bqT if g < NDB else bkT)
                src = QT16 if g < NDB else KT16
                vi = g % NDB
                ps = mp.tile([P, S], F32, tag="mp", name="psB")
                for dj in range(NDB):
                    nc.tensor.matmul(
                        ps[:],
                        W16[wname][:, dj, vi * P:(vi + 1) * P],
                        src[dj][:],
                        start=(dj == 0),
                        stop=(dj == NDB - 1),
                    )
                t = projp.tile([P, S], BF16, tag="projT", name="projT")
                # qT and kT drains on ACT (DVE is the busier engine)
                if bT is None:
                    nc.scalar.activation(t[:], ps[:], AF.Copy)
                else:
                    nc.scalar.activation(t[:], ps[:], AF.Identity,
                                         bias=bT[:, vi:vi + 1])
                PROJ[b].setdefault("qT" if g < NDB else "kT", []).append(t)
            else:      # v groups (g 8-11)
                si = g - 8
                ps = mp.tile([P, S], F32, tag="mp", name="psV")
                for dj in range(NDB):
                    nc.tensor.matmul(
                        ps[:],
                        KT16[dj][:, si * P:(si + 1) * P],
                        W16["Wv"][:, dj, :],
                        start=(dj == 0),
                        stop=(dj == NDB - 1),
                    )
                if VAUG[b] is None:
                    VAUG[b] = []
                va = vaugp.tile([P, H, DH + 1], BF16, tag="vaug", name="vaug")
                nc.vector.memset(va[:, :, DH:DH + 1], 1.0)
                if zero_bias:
                    nc.vector.tensor_copy(
                        va[:, :, 0:DH], ps.rearrange("p (h d) -> p h d", h=H)
                    )
                else:
                    nc.vector.tensor_tensor(
                        va[:, :, 0:DH],
                        ps.rearrange("p (h d) -> p h d", h=H),
                        bv_bc.rearrange("p (h d) -> p h d", h=H),
                        OP.add,
                    )
                VAUG[b].append(va)

        def emit_qnat_group(b, si):
            qT16 = PROJ[b]["qT"]
            ps = mp.tile([P, S], BF16, tag="mp", name="psQn")
            for vi in range(NDB):
                nc.tensor.transpose(
                    ps[:, vi * P:(vi + 1) * P],
                    qT16[vi][:, si * P:(si + 1) * P],
                    I128b[:],
                )
            if QNAT[b] is None:
                QNAT[b] = []
            qn = qnatp.tile([P, S], BF16, tag="qnat", name="qnat")
            nc.vector.tensor_copy(qn[:], ps[:])
            QNAT[b].append(qn)

        def emit_C_pair(b, hp):
            # heads (2hp, 2hp+1) share feature block hp; per ki both heads'
            # score matmuls -> one [128, 2, 512] PSUM tile, one exp, then both
            # heads' A@V accumulations for that ki.
            qT16, kT16 = PROJ[b]["qT"], PROJ[b]["kT"]
            if SUMS[b] is None:
                SUMS[b] = dramp.tile([H, S], BF16, tag="sums", name="sums")
            vi = hp
            ea = expp.tile([P, 2, NBLK, S], BF16, tag="expA", name="expA")
            pos = [None, None]
            for ki in range(NBLK):
                ps = scp.tile([P, 2, S], F32, tag="scp", name="scp")
                for u in range(2):
                    hof = u * DH
                    nc.tensor.matmul(
                        ps[:, u, :],
                        kT16[vi][hof:hof + DH, ki * P:(ki + 1) * P],
                        qT16[vi][hof:hof + DH, :],
                        start=True,
                        stop=True,
                    )
                nc.scalar.activation(
                    ea[:, :, ki, :], ps[:], AF.Exp, scale=SC,
                )
                for u in range(2):
                    h = 2 * hp + u
                    if ki == 0:
                        pos[u] = pop.tile([P, S], F32, tag="po", name="po")
                    nc.tensor.matmul(
                        pos[u][0:DH + 1, :],
                        VAUG[b][ki][:, h, :],
                        ea[:, u, ki, :],
                        start=(ki == 0),
                        stop=(ki == NBLK - 1),
                    )
            for u in range(2):
                h = 2 * hp + u
                at = atp.tile([DH + 1, S], BF16, tag="at", name="at")
                nc.vector.tensor_copy(at[:], pos[u][0:DH + 1, :])
                nc.sync.dma_start(SUMS[b][h:h + 1, :], at[DH:DH + 1, :])
                AT[b][h] = at

        def emit_sum_gathers(b):
            RN[b] = []
            for si in range(NBLK):
                sg = rnp.tile([P, H], BF16, tag="sg", name="sg", bufs=5)
                nc.sync.dma_start(
                    sg[:],
                    SUMS[b][:, si * P:(si + 1) * P].rearrange("h s -> s h"),
                )
                rn = rnp.tile([P, H], F32, tag="rn", name="rn", bufs=8)
                nc.vector.reciprocal(rn[:], sg[:])
                RN[b].append(rn)

        def emit_D_group(b, si, last):
            rn = RN[b][si]
            pa = mp.tile([P, S], BF16, tag="mp", name="psD")
            for h in range(H):
                nc.tensor.transpose(
                    pa[:, h * DH:(h + 1) * DH],
                    AT[b][h][0:DH, si * P:(si + 1) * P],
                    I128b[0:DH, 0:DH],
                )
            if OH[b] is None:
                OH[b] = []
            o = ohp.tile([P, D], BF16, tag="oh", name="oh")
            nc.vector.tensor_tensor(
                o.rearrange("p (h d) -> p h d", h=H),
                pa.rearrange("p (h d) -> p h d", h=H),
                rn[:, :, None].to_broadcast((P, H, DH)),
                OP.mult,
            )
            if last:
                nc.vector.tensor_tensor(o[:], o[:], QNAT[b][si][:], OP.add)
            else:
                nc.gpsimd.tensor_tensor(o[:], o[:], QNAT[b][si][:], OP.add)
            OH[b].append(o)

        def emit_E(b):
            g0 = None if unit_ln else gbc["ln0_g"]
            b0 = None if unit_ln else gbc["ln0_b"]
            LN0[b] = []
            mv4, istd4 = ln_stats(OH[b])
            for si in range(NBLK):
                dst = ln0p.tile([P, D], BF16, tag="ln0", name="ln0")
                ln_apply_into(dst[:], OH[b][si], mv4, istd4, si, g0, b0)
                LN0[b].append(dst)

        def emit_F_lnT(b, vi):
            ps = mp.tile([P, S], BF16, tag="mp", name="psF")
            for si in range(NBLK):
                nc.tensor.transpose(
                    ps[:, si * P:(si + 1) * P],
                    LN0[b][si][:, vi * P:(vi + 1) * P],
                    I128b[:],
                )
            if LNT[b] is None:
                LNT[b] = []
            t = lntp.tile([P, S], BF16, tag="lnT", name="lnT")
            nc.vector.tensor_copy(t[:], ps[:])
            LNT[b].append(t)

        def emit_F_fc(b, si):
            ps = mp.tile([P, S], F32, tag="mp", name="psFc")
            for dj in range(NDB):
                nc.tensor.matmul(
                    ps[:],
                    LNT[b][dj][:, si * P:(si + 1) * P],
                    W16["Wo"][:, dj, :],
                    start=(dj == 0),
                    stop=(dj == NDB - 1),
                )
            if PRE2[b] is None:
                PRE2[b] = []
            p2 = p2p.tile([P, D], BF16, tag="pre2", name="pre2")
            if zero_bias:
                # p2 = relu(fc) + ln0 fused: (ps max 0) + ln0
                nc.vector.scalar_tensor_tensor(
                    p2[:], ps[:], 0.0, LN0[b][si][:], OP.max, OP.add
                )
            else:
                tmp = statp.tile([P, D], F32, tag="fcb", name="fcb")
                nc.vector.tensor_tensor(tmp[:], ps[:], bo_bc[:], OP.add)
                rl = statp.tile([P, D], BF16, tag="relu", name="relu")
                nc.scalar.activation(rl[:], tmp[:], AF.Relu)
                nc.vector.tensor_tensor(p2[:], rl[:], LN0[b][si][:], OP.add)
            PRE2[b].append(p2)

        def emit_E2(b):
            g1 = None if unit_ln else gbc["ln1_g"]
            b1 = None if unit_ln else gbc["ln1_b"]
            mv4b, istd4b = ln_stats(PRE2[b])
            of = outp.tile([P, NBLK, D], F32, tag="outf", name="outf")
            for si in range(NBLK):
                ln_apply_into(of[:, si, :], PRE2[b][si], mv4b, istd4b, si,
                              g1, b1)
            nc.sync.dma_start(
                Od[b].rearrange("(si p) d -> p si d", p=P), of[:]
            )

        # ---- staged emission: 3 batches in flight ----
        def tail_pieces(b):
            last = (b == NB - 1)
            th = []
            for si in range(NBLK):
                th.append(lambda si=si: emit_D_group(b, si, last))
            th.append(lambda: emit_E(b))
            for vi in range(NDB):
                th.append(lambda vi=vi: emit_F_lnT(b, vi))
            for si in range(NBLK):
                th.append(lambda si=si: emit_F_fc(b, si))
            return th  # 13 pieces; E2 emitted separately (post-pairs)

        def prep_pieces(nb):
            th = []
            for g in range(2 * NDB):
                th.append(lambda g=g: emit_A_group(nb, g))
            for g in range(12):
                th.append(lambda g=g: emit_B_group(nb, g))
            return th  # 20 pieces; qnat emitted post-pairs

        # prologue: batch-0 path to first PE work
        emit_load(0, "Q")
        emit_weight_load("Wq", 0)
        emit_load(0, "K")
        emit_weight_load("Wk", 1)
        emit_weight_load("Wv", 2)
        emit_weight_load("Wo", 3)
        for th in prep_pieces(0):
            th()
        for si in range(NBLK):
            emit_qnat_group(0, si)
        emit_load(1, "Q")
        emit_load(1, "K")

        for b in range(NB):
            nb = b + 1
            tails = tail_pieces(b - 1) if b > 0 else []
            preps = prep_pieces(nb) if nb < NB else []
            # A-groups first so their PSUM drains lead the DVE queue (keeps
            # the mp ring moving), then D/E, then B chains, then lnT/fc
            inter = preps[:8] + tails[:5] + preps[8:] + tails[5:]
            n_per = (len(inter) + 3) // 4 if inter else 0
            fi = 0
            for hp in range(H // 2):
                emit_C_pair(b, hp)
                if hp == H // 2 - 1:
                    emit_sum_gathers(b)
                for _ in range(n_per):
                    if fi < len(inter):
                        inter[fi]()
                        fi += 1
            while fi < len(inter):
                inter[fi]()
                fi += 1
            if b > 0:
                emit_E2(b - 1)
            if nb < NB:
                for si in range(NBLK):
                    emit_qnat_group(nb, si)
            if b + 2 < NB:
                emit_load(b + 2, "Q")
                emit_load(b + 2, "K")

        # epilogue: last batch tail, fully serial
        for th in tail_pieces(NB - 1):
            th()
        emit_E2(NB - 1)


_CACHE = {}


def _get_program(zero_bias: bool, unit_ln: bool):
    key = (zero_bias, unit_ln)
    if key not in _CACHE:
        _CACHE[key] = build_program(zero_bias, unit_ln)
    return _CACHE[key]


def _make_in_maps(inputs):
    Q = np.ascontiguousarray(inputs["Q"], dtype=np.float32)
    K = np.ascontiguousarray(inputs["K"], dtype=np.float32)
    shared = {
        name: np.ascontiguousarray(inputs[name], dtype=np.float32)
        for name in ("Wq", "Wk", "Wv", "Wo", "bq", "bk", "bv", "bo",
                     "ln0_g", "ln0_b", "ln1_g", "ln1_b")
    }
    in_maps = []
    for c in range(NCORES):
        m = dict(shared)
        m["Q"] = Q[c * NB:(c + 1) * NB]
        m["K"] = K[c * NB:(c + 1) * NB]
        in_maps.append(m)
    return in_maps


def run(inputs, trace=False):
    zero_bias = all(
        not np.any(inputs[v]) for v in ("bq", "bk", "bv", "bo")
    )
    unit_ln = (
        np.all(inputs["ln0_g"] == 1.0) and np.all(inputs["ln1_g"] == 1.0)
        and not np.any(inputs["ln0_b"]) and not np.any(inputs["ln1_b"])
    )
    nc = _get_program(zero_bias, unit_ln)
    res = run_bass_kernel_spmd(
        nc, _make_in_maps(inputs), core_ids=list(range(NCORES)), trace=trace
    )
    out = np.concatenate([res.results[c]["out"] for c in range(NCORES)], axis=0)
    return out, res


def kernel(**inputs):
    B, Sq, Dq = inputs["Q"].shape
    assert (B, Sq, Dq) == (NB * NCORES, S, D), (B, Sq, Dq)
    out, _ = run(inputs, trace=False)
    return out
